# revision 12
# baseline (speedup 1.0000x reference)
"""Trainium2 Bass kernel for nn_PhenoCapsuleHead (capsule head with EM routing).

Contract: kernel(**inputs) takes the FULL inputs (B=4096) and returns the full
6-tuple (class_logits, act, pose, q, primary_pose, primary_act), matching
reference.reference(). Internally shards batch-parallel over 8 NeuronCores.

Design (per core, Bc=512, 4 partition-tiles of 128 samples):
  phase 1  : big matmul z_ext @ W_ext on PE (fp32r), where W_ext packs
             [W_prim | ln1_g*W_act | ones/D] and a ones-row folds in all biases.
             LayerNorm(z) @ W_act is computed analytically from (z@Wg, mean, var);
             sum(z^2) via ACT Square + PE ones-column matmuls.
  phase 1.5: EM init pose0 = einsum(a/25 * pose_p, W_caps) done entirely on PE.
  phase 2  : votes = pose_p @ W_caps per input-capsule (K=32 row-packed matmuls),
             EM routing on VectorE in fp16 (m padded 25->26 for 2x mode),
             with in-place tree reductions; exp/sigmoid on ScalarE.
"""
import os
import numpy as np

import concourse.bass as bass
import concourse.bacc as bacc
import concourse.tile as tile
from concourse import mybir
from concourse.bass_utils import run_bass_kernel_spmd

F32 = mybir.dt.float32
F32R = mybir.dt.float32r
F16 = mybir.dt.float16
AF = mybir.ActivationFunctionType
ALU = mybir.AluOpType
AX = mybir.AxisListType

N_CORES = 8
B = 4096
BC = B // N_CORES          # 512 per core
P = 128
NT = BC // P               # 4 partition tiles per core
D = 4096
KC = D // P                # 32 contraction chunks
NP = 64                    # primary capsules
DP = 32                    # primary dim
O = 16                     # class capsule dim
M = 25                     # n classes
MP = 26                    # padded (even, 4B-aligned fp16 runs)
OM = O * MP                # 416
NW = 2048 + NP + 4         # 2116 columns of W_ext (aligned for fp32r)
NCH = [512, 512, 512, 512, NW - 2048]   # N-chunks of the big matmul
EPS = 1e-8
LN_EPS = 1e-5
SCALE = float(1.0 / np.sqrt(O))

ROUTE16 = os.environ.get("ROUTE_F32", "0") != "1"   # fp16 routing by default
PHASE_LIMIT = os.environ.get("PHASE_LIMIT", "full")  # '1', '15', 'votes', 'full'
RDT = F16 if ROUTE16 else F32


def _bc(ap_obj, idx, count):
    """Insert a 0-step (broadcast) dim at position idx of an AP (0 = partition dim)."""
    new = [list(p) for p in ap_obj.ap]
    new.insert(idx, [0, count])
    return bass.AP(tensor=ap_obj.tensor, offset=ap_obj.offset, ap=new)


def _r(ap_obj):
    return ap_obj.bitcast(F32R)


def build_program():
    nc = bacc.Bacc("TRN2", target_bir_lowering=False, debug=False,
                   num_devices=N_CORES)

    zt = nc.dram_tensor("zt", [D + 1, BC], F32R, kind="ExternalInput").ap()
    wext = nc.dram_tensor("wext", [D + 1, NW], F32R, kind="ExternalInput").ap()
    wcaps = nc.dram_tensor("wcaps", [P, 16, OM], F16, kind="ExternalInput").ap()
    rrep = nc.dram_tensor("rrep", [NP, 16, P], F16, kind="ExternalInput").ap()
    identw = nc.dram_tensor("identw", [P, P], F32, kind="ExternalInput").ap()
    c1rep = nc.dram_tensor("c1rep", [P, NP], F32, kind="ExternalInput").ap()
    wcombrep = nc.dram_tensor("wcombrep", [P, O], F32, kind="ExternalInput").ap()
    cbeta = nc.dram_tensor("cbeta", [P, 1], F32, kind="ExternalInput").ap()

    o_ppose = nc.dram_tensor("ppose", [BC, 2048], F32, kind="ExternalOutput").ap()
    o_pact = nc.dram_tensor("pact", [BC, NP], F32, kind="ExternalOutput").ap()
    o_q = nc.dram_tensor("q", [BC, NP, M], F32, kind="ExternalOutput").ap()
    o_pose = nc.dram_tensor("pose", [BC, M, O], F32, kind="ExternalOutput").ap()
    o_act = nc.dram_tensor("act", [BC, M], F32, kind="ExternalOutput").ap()
    o_cls = nc.dram_tensor("cls", [BC, M], F32, kind="ExternalOutput").ap()

    with tile.TileContext(nc) as tc:
        _build(nc, tc, zt, wext, wcaps, rrep, identw, c1rep, wcombrep, cbeta,
               o_ppose, o_pact, o_q, o_pose, o_act, o_cls)
    nc.compile()
    return nc


def _build(nc, tc, zt, wext, wcaps, rrep, identw, c1rep, wcombrep, cbeta,
           o_ppose, o_pact, o_q, o_pose, o_act, o_cls):
    from contextlib import ExitStack
    ctx = ExitStack()
    with ctx:
        const = ctx.enter_context(tc.tile_pool(name="const", bufs=1))
        persist = ctx.enter_context(tc.tile_pool(name="persist", bufs=1))
        smalls = ctx.enter_context(tc.tile_pool(name="smalls", bufs=4))

        # ---------------- constants ----------------
        wcaps_s = const.tile([P, 16, OM], F16)
        nc.sync.dma_start(out=wcaps_s, in_=wcaps)
        rrep_s = const.tile([NP, 16, P], F16)
        nc.sync.dma_start(out=rrep_s, in_=rrep)
        ident_s = const.tile([P, P], F32)
        nc.sync.dma_start(out=ident_s, in_=identw)
        c1rep_s = const.tile([P, NP], F32)
        nc.sync.dma_start(out=c1rep_s, in_=c1rep)
        wcomb_s = const.tile([P, O], F32)
        nc.sync.dma_start(out=wcomb_s, in_=wcombrep)
        cbeta_s = const.tile([P, 1], F32)
        nc.sync.dma_start(out=cbeta_s, in_=cbeta)
        ones_col = const.tile([P, 1], F32)
        nc.vector.memset(ones_col, 1.0)
        lneps_t = const.tile([P, 1], F32)
        nc.vector.memset(lneps_t, LN_EPS)
        ztb = const.tile([1, BC], F32R)
        nc.sync.dma_start(out=ztb, in_=zt[D:D + 1, :])

        # ---------------- persistent state ----------------
        pose_pT = persist.tile([P, 16, BC], F16)       # [nd, c, b] 16KB/part
        a_full = persist.tile([P, NT, NP], F32)        # primary_act per tile
        zwg_s = persist.tile([P, NT, NP], F32)
        mu_s = persist.tile([P, NT, 1], F32)
        rr_s = persist.tile([P, NT, 1], F32)
        aT_s = persist.tile([NP, BC], F16)             # a transposed [n, b]
        pose_f32 = [persist.tile([P, O, MP], F32, tag=f"pose{t}", name=f"pose_f32_{t}")
                    for t in range(NT)]

        # ================= phase 1: big matmul =================
        with tc.tile_pool(name="ztp", bufs=1) as ztp, \
             tc.tile_pool(name="stream", bufs=3) as stream, \
             tc.tile_pool(name="zsqp", bufs=16) as zsqp, \
             tc.tile_pool(name="stage1", bufs=1) as stage1:
            zt_s = ztp.tile([P, KC, BC], F32R)          # 64KB/part, freed after ph.1
            for kc in range(KC):
                nc.sync.dma_start(out=zt_s[:, kc, :], in_=zt[P * kc:P * (kc + 1), :])
            pose_ps = stage1.tile([P, NT, 2048], F32)   # 32KB/part staging

            with tc.tile_pool(name="bigp", bufs=2, space="PSUM") as bigp:
                ncol0 = 0
                for nci, nsz in enumerate(NCH):
                    bps = [bigp.tile([P, 512], F32, tag=f"big{bt}", name=f"bps{bt}")
                           for bt in range(NT)]
                    for kc in range(KC):
                        wt = stream.tile([P, 512], F32R, tag="wt")
                        nc.sync.dma_start(
                            out=wt[:, 0:nsz],
                            in_=wext[P * kc:P * (kc + 1), ncol0:ncol0 + nsz])
                        for bt in range(NT):
                            nc.tensor.matmul(bps[bt][:, 0:nsz],
                                             zt_s[:, kc, P * bt:P * (bt + 1)],
                                             wt[:, 0:nsz],
                                             start=(kc == 0), stop=False)
                    wb = stream.tile([1, 512], F32R, tag="wb")
                    nc.sync.dma_start(out=wb[:, 0:nsz],
                                      in_=wext[D:D + 1, ncol0:ncol0 + nsz])
                    for bt in range(NT):
                        nc.tensor.matmul(bps[bt][:, 0:nsz],
                                         ztb[:, P * bt:P * (bt + 1)],
                                         wb[:, 0:nsz], start=False, stop=True)
                    for bt in range(NT):
                        if nci < 4:
                            nc.scalar.activation(
                                out=pose_ps[:, bt, ncol0:ncol0 + nsz],
                                in_=bps[bt], func=AF.Copy)
                            nc.sync.dma_start(
                                out=o_ppose[P * bt:P * (bt + 1), ncol0:ncol0 + nsz],
                                in_=pose_ps[:, bt, ncol0:ncol0 + nsz])
                        else:
                            nc.scalar.activation(out=zwg_s[:, bt, :],
                                                 in_=bps[bt][:, 0:NP], func=AF.Copy)
                            nc.vector.tensor_copy(mu_s[:, bt, :],
                                                  bps[bt][:, NP:NP + 1])
                    ncol0 += nsz

            # ---- pose_p transposes + sum(z^2) + primary_act (bigp freed) ----
            with tc.tile_pool(name="ssp", bufs=1, space="PSUM") as ssp, \
                 tc.tile_pool(name="tp_ps", bufs=2, space="PSUM") as tp_ps, \
                 tc.tile_pool(name="tp_ps2", bufs=2, space="PSUM") as tp_ps2:
                for cc in range(16):
                    for bt in range(NT):
                        tp = tp_ps.tile([P, P], F32, tag="tp")
                        nc.tensor.transpose(
                            tp, pose_ps[:, bt, P * cc:P * (cc + 1)], ident_s)
                        nc.vector.tensor_copy(
                            pose_pT[:, cc, P * bt:P * (bt + 1)], tp)
                ssb = [ssp.tile([P, 1], F32, tag=f"ss{bt}", name=f"ssb{bt}") for bt in range(NT)]
                for kc in range(KC):
                    zsq = zsqp.tile([P, 512], F32, tag="zsq")
                    nc.scalar.activation(out=zsq, in_=zt_s[:, kc, :].bitcast(F32),
                                         func=AF.Square)
                    for bt in range(NT):
                        nc.tensor.matmul(ssb[bt], zsq[:, P * bt:P * (bt + 1)],
                                         ones_col, start=(kc == 0),
                                         stop=(kc == KC - 1))
                for bt in range(NT):
                    musq = smalls.tile([P, 1], F32, tag="musq")
                    nc.vector.tensor_mul(musq, mu_s[:, bt, :], mu_s[:, bt, :])
                    var = smalls.tile([P, 1], F32, tag="var")
                    nc.vector.scalar_tensor_tensor(
                        out=var, in0=ssb[bt], scalar=1.0 / D, in1=musq,
                        op0=ALU.mult, op1=ALU.subtract)
                    sd = smalls.tile([P, 1], F32, tag="sd")
                    nc.scalar.activation(out=sd, in_=var, func=AF.Sqrt, bias=lneps_t)
                    nc.vector.reciprocal(rr_s[:, bt, :], sd)
                    rmun = smalls.tile([P, 1], F32, tag="rmun")
                    nc.vector.tensor_mul(rmun, rr_s[:, bt, :], mu_s[:, bt, :])
                    nc.vector.tensor_scalar_mul(rmun, rmun, -1.0)
                    actl = smalls.tile([P, NP], F32, tag="actl")
                    nc.vector.tensor_scalar_mul(actl, zwg_s[:, bt, :], rr_s[:, bt, :])
                    nc.vector.scalar_tensor_tensor(
                        out=actl, in0=c1rep_s, scalar=rmun, in1=actl,
                        op0=ALU.mult, op1=ALU.add)
                    nc.scalar.activation(out=a_full[:, bt, :], in_=actl,
                                         func=AF.Sigmoid)
                    nc.sync.dma_start(out=o_pact[P * bt:P * (bt + 1), :],
                                      in_=a_full[:, bt, :])
                    tpa = tp_ps2.tile([NP, P], F32, tag="tpa")
                    nc.tensor.transpose(tpa, a_full[:, bt, :], ident_s)
                    nc.scalar.activation(out=aT_s[:, P * bt:P * (bt + 1)], in_=tpa,
                                         func=AF.Copy)

        if PHASE_LIMIT == "1":
            return
        # ================= phase 1.5: init pose0 via PE =================
        with tc.tile_pool(name="p15", bufs=3) as p15, \
             tc.tile_pool(name="p15ps", bufs=2, space="PSUM") as p15ps, \
             tc.tile_pool(name="p0acc", bufs=1, space="PSUM") as p0acc:
            omch = [(0, 128), (128, 128), (256, 128), (384, OM - 384)]
            pacc = [p0acc.tile([oc[1], BC], F32, tag=f"p0{j}", name=f"pacc{j}")
                    for j, oc in enumerate(omch)]
            for c16 in range(16):
                rep = p15ps.tile([P, BC], F32, tag="rep")
                nc.tensor.matmul(rep, rrep_s[:, c16, :], aT_s,
                                 start=True, stop=True)
                aps = p15.tile([P, BC], F16, tag="aps")
                nc.vector.tensor_mul(aps, pose_pT[:, c16, :], rep)
                for j, (o0, osz) in enumerate(omch):
                    nc.tensor.matmul(pacc[j], wcaps_s[:, c16, o0:o0 + osz],
                                     aps, start=(c16 == 0), stop=(c16 == 15))
            for j, (o0, osz) in enumerate(omch):
                pst = p15.tile([P, BC], F32, tag="pst")
                nc.scalar.activation(out=pst[0:osz, :], in_=pacc[j], func=AF.Copy)
                for bt in range(NT):
                    tp = p15ps.tile([P, P], F32, tag="tp15")
                    nc.tensor.transpose(tp[:, 0:osz],
                                        pst[0:osz, P * bt:P * (bt + 1)],
                                        ident_s[0:osz, 0:osz])
                    pf = pose_f32[bt]
                    nc.scalar.activation(
                        out=bass.AP(tensor=pf.tensor, offset=pf.offset + o0,
                                    ap=[list(pf.ap[0]), [1, osz]]),
                        in_=tp[:, 0:osz], func=AF.Copy)

        if PHASE_LIMIT == "15":
            return
        # ================= phase 2: votes + routing =================
        # V and tmp are split in n-halves over a 5-slot rotation so the next
        # tile's votes copies overlap the current tile's routing.
        with tc.tile_pool(name="bigbuf", bufs=5) as bigbuf, \
             tc.tile_pool(name="rt", bufs=1) as rt, \
             tc.tile_pool(name="rt2", bufs=2) as rt2, \
             tc.tile_pool(name="vps", bufs=2, space="PSUM") as vps:
            HN = NP // 2
            for bt in range(NT):
                # ---- votes ----
                Va = bigbuf.tile([P, HN, O, MP], RDT, tag="hv", name=f"Va{bt}")
                Vb = bigbuf.tile([P, HN, O, MP], RDT, tag="hv", name=f"Vb{bt}")
                for c16 in range(16):
                    ps = vps.tile([P, 4, 512], F32, tag="v")
                    for r in range(4):
                        nc.tensor.matmul(
                            ps[:, r, 0:OM],
                            pose_pT[DP * r:DP * (r + 1), c16, P * bt:P * (bt + 1)],
                            wcaps_s[DP * r:DP * (r + 1), c16, :],
                            start=True, stop=True, tile_position=(DP * r, 0))
                    half = Va if c16 < 8 else Vb
                    c4 = 4 * (c16 % 8)
                    nc.scalar.activation(
                        out=half[:, c4:c4 + 4, :, :].rearrange(
                            "p a o m -> p a (o m)"),
                        in_=ps[:, :, 0:OM], func=AF.Copy)

                if PHASE_LIMIT == "votes":
                    continue
                # ---- routing state ----
                a_t = a_full[:, bt, :]
                actE = rt.tile([P, MP], F32, tag="actE")
                suma = smalls.tile([P, 1], F32, tag="suma")
                nc.vector.reduce_sum(suma, a_t, axis=AX.X)
                act01 = smalls.tile([P, 1], F32, tag="act01")
                nc.scalar.activation(out=act01, in_=suma, func=AF.Sigmoid,
                                     scale=1.0 / M)
                nc.vector.tensor_scalar_add(
                    actE, bass.AP(tensor=act01.tensor, offset=act01.offset,
                                  ap=[list(act01.ap[0]), [0, MP]]), EPS)
                nc.vector.memset(actE[:, M:MP], 0.0)

                ta = bigbuf.tile([P, HN, O, MP], RDT, tag="hv", name=f"ta{bt}")
                tb = bigbuf.tile([P, HN, O, MP], RDT, tag="hv", name=f"tb{bt}")
                posef = rt.tile([P, O, MP], RDT, tag="posef")
                if ROUTE16:
                    nc.vector.tensor_copy(posef, pose_f32[bt])
                e_s = rt2.tile([P, NP, MP], F32, tag="e")
                qsum = rt.tile([P, MP], F32, tag="qsum")
                act_s = rt2.tile([P, MP], F32, tag="acts")
                Z = rt.tile([P, NP], F32, tag="Z")
                rz = rt.tile([P, NP], F32, tag="rz")

                for it in range(2):
                    psrc = posef if ROUTE16 else pose_f32[bt]
                    dot = rt.tile([P, NP, MP], RDT, tag="dq")
                    for V, tmp, n0 in ((Va, ta, 0), (Vb, tb, HN)):
                        nc.vector.tensor_mul(tmp, V, _bc(psrc, 1, HN))
                        nc.vector.tensor_add(tmp[:, :, 0:8, :], tmp[:, :, 0:8, :],
                                             tmp[:, :, 8:16, :])
                        nc.vector.tensor_add(tmp[:, :, 0:4, :], tmp[:, :, 0:4, :],
                                             tmp[:, :, 4:8, :])
                        nc.vector.tensor_add(tmp[:, :, 0:2, :], tmp[:, :, 0:2, :],
                                             tmp[:, :, 2:4, :])
                        nc.vector.tensor_add(dot[:, n0:n0 + HN, :],
                                             tmp[:, :, 0, :], tmp[:, :, 1, :])
                    nc.scalar.activation(out=e_s, in_=dot, func=AF.Exp, scale=SCALE)
                    nc.vector.tensor_mul(e_s, e_s, _bc(actE, 1, NP))
                    nc.vector.reduce_sum(Z, e_s, axis=AX.X)
                    nc.vector.reciprocal(rz, Z)
                    nc.vector.tensor_mul(e_s, e_s, _bc(rz, 2, MP))   # q (f32)
                    qa = rt.tile([P, NP, MP], RDT, tag="dq")
                    nc.vector.tensor_mul(qa, e_s, _bc(a_t, 2, MP))
                    nc.vector.reduce_sum(qsum, qa.rearrange("p n m -> p m n"),
                                         axis=AX.X)
                    nc.scalar.activation(out=act_s, in_=qsum, func=AF.Sigmoid)
                    if it == 0:
                        nc.vector.tensor_scalar_add(actE, act_s, EPS)
                        nc.vector.memset(actE[:, M:MP], 0.0)
                    nc.vector.tensor_mul(ta, Va, _bc(qa[:, 0:HN, :], 2, O))
                    nc.vector.tensor_mul(tb, Vb, _bc(qa[:, HN:NP, :], 2, O))
                    nc.vector.tensor_add(ta, ta, tb)
                    nc.vector.tensor_add(ta[:, 0:16], ta[:, 0:16], ta[:, 16:32])
                    nc.vector.tensor_add(ta[:, 0:8], ta[:, 0:8], ta[:, 8:16])
                    nc.vector.tensor_add(ta[:, 0:4], ta[:, 0:4], ta[:, 4:8])
                    nc.vector.tensor_add(ta[:, 0:2], ta[:, 0:2], ta[:, 2:4])
                    nc.vector.tensor_add(pose_f32[bt], ta[:, 0], ta[:, 1])
                    if it == 0 and ROUTE16:
                        nc.vector.tensor_copy(posef, pose_f32[bt])

                # ---- outputs of routing ----
                nc.sync.dma_start(out=o_q[P * bt:P * (bt + 1), :, :],
                                  in_=e_s[:, :, 0:M])
                nc.sync.dma_start(out=o_act[P * bt:P * (bt + 1), :],
                                  in_=act_s[:, 0:M])
                pmo = rt2.tile([P, M, O], F32, tag="pmo")
                nc.vector.tensor_copy(
                    pmo.rearrange("p m o -> p o m"), pose_f32[bt][:, :, 0:M])
                nc.sync.dma_start(out=o_pose[P * bt:P * (bt + 1), :, :], in_=pmo)

                # ---- class logits: LN over o ----
                pf = pose_f32[bt]
                mu2 = rt.tile([P, MP], F32, tag="mu2")
                nc.vector.tensor_reduce(mu2, pf.rearrange("p o m -> p m o"),
                                        axis=AX.X, op=ALU.add)
                dctr = rt.tile([P, O, MP], F32, tag="dctr")
                nc.vector.scalar_tensor_tensor(
                    out=dctr, in0=_bc(mu2, 1, O), scalar=-1.0 / O, in1=pf,
                    op0=ALU.mult, op1=ALU.add)
                sq2 = rt.tile([P, O, MP], F32, tag="sqtw")
                nc.vector.tensor_mul(sq2, dctr, dctr)
                v2 = rt.tile([P, MP], F32, tag="v2")
                nc.vector.tensor_reduce(v2, sq2.rearrange("p o m -> p m o"),
                                        axis=AX.X, op=ALU.add)
                sd2 = rt.tile([P, MP], F32, tag="sd2")
                nc.scalar.activation(out=sd2, in_=v2, func=AF.Sqrt, bias=lneps_t,
                                     scale=1.0 / O)
                r2 = rt.tile([P, MP], F32, tag="r2")
                nc.vector.reciprocal(r2, sd2)
                tw = rt.tile([P, O, MP], F32, tag="sqtw")
                nc.vector.tensor_mul(tw, dctr, _bc(wcomb_s, 2, MP))
                inner = rt.tile([P, MP], F32, tag="inner")
                nc.vector.tensor_reduce(inner, tw.rearrange("p o m -> p m o"),
                                        axis=AX.X, op=ALU.add)
                cls = rt2.tile([P, MP], F32, tag="cls")
                nc.vector.tensor_mul(cls, inner, r2)
                nc.vector.tensor_scalar_add(
                    cls, cls, bass.AP(tensor=cbeta_s.tensor, offset=cbeta_s.offset,
                                      ap=[list(cbeta_s.ap[0]), [1, 1]]))
                nc.sync.dma_start(out=o_cls[P * bt:P * (bt + 1), :], in_=cls[:, 0:M])


def prepare_host(inputs):
    """Host-side weight preparation (shared across cores)."""
    z = np.ascontiguousarray(np.asarray(inputs["z"], dtype=np.float32))
    W_prim = np.asarray(inputs["W_prim"], np.float32)
    b_prim = np.asarray(inputs["b_prim"], np.float32)
    ln1_g = np.asarray(inputs["ln1_g"], np.float32)
    ln1_b = np.asarray(inputs["ln1_b"], np.float32)
    W_act = np.asarray(inputs["W_act"], np.float32)
    b_act = np.asarray(inputs["b_act"], np.float32)
    W_caps = np.asarray(inputs["W_caps"], np.float32)
    ln2_g = np.asarray(inputs["ln2_g"], np.float32)
    ln2_b = np.asarray(inputs["ln2_b"], np.float32)
    W_out = np.asarray(inputs["W_out"], np.float32)
    b_out = np.asarray(inputs["b_out"], np.float32)

    W_ext = np.zeros((D + 1, NW), np.float32)
    W_ext[:D, :2048] = W_prim
    W_ext[:D, 2048:2112] = ln1_g[:, None] * W_act
    W_ext[:D, 2112] = 1.0 / D
    W_ext[D, :2048] = b_prim
    W_ext[D, 2048:2112] = ln1_b @ W_act + b_act
    W_ext[D, 2112] = 0.0

    W_caps2 = np.zeros((NP, DP, O, MP), np.float32)
    W_caps2[:, :, :, :M] = np.transpose(W_caps, (0, 1, 3, 2))
    wcaps_arr = np.ascontiguousarray(
        W_caps2.reshape(16, 4, DP, OM).transpose(1, 2, 0, 3).reshape(P, 16, OM)
    ).astype(np.float16)

    rrep_arr = np.zeros((NP, 16, P), np.float16)
    for n in range(NP):
        c, r = n // 4, n % 4
        rrep_arr[n, c, DP * r:DP * (r + 1)] = 1.0 / M

    ident = np.eye(P, dtype=np.float32)
    c1 = W_act.T @ ln1_g
    c1rep_arr = np.ascontiguousarray(np.broadcast_to(c1, (P, NP)))
    wcomb = ln2_g * W_out[:, 0]
    wcomb_arr = np.ascontiguousarray(np.broadcast_to(wcomb, (P, O)))
    cb = np.float32(ln2_b @ W_out[:, 0] + b_out[0])
    cbeta_arr = np.full((P, 1), cb, np.float32)

    zt_full = np.concatenate([z.T, np.ones((1, B), np.float32)], axis=0)  # [4097, B]
    shared = dict(wext=W_ext, wcaps=wcaps_arr, rrep=rrep_arr, identw=ident,
                  c1rep=c1rep_arr, wcombrep=wcomb_arr, cbeta=cbeta_arr)
    in_maps = []
    for c in range(N_CORES):
        m = dict(shared)
        m["zt"] = np.ascontiguousarray(zt_full[:, c * BC:(c + 1) * BC])
        in_maps.append(m)
    return in_maps


_CACHE = {}


def _get_program():
    key = ("nc_f16" if ROUTE16 else "nc_f32") + PHASE_LIMIT
    if key not in _CACHE:
        _CACHE[key] = build_program()
    return _CACHE[key]


def assemble(results):
    cls = np.concatenate([r["cls"] for r in results], axis=0)
    act = np.concatenate([r["act"] for r in results], axis=0)
    pose = np.concatenate([r["pose"] for r in results], axis=0)
    q = np.concatenate([r["q"] for r in results], axis=0)
    ppose = np.concatenate([r["ppose"] for r in results], axis=0)
    pact = np.concatenate([r["pact"] for r in results], axis=0)
    return (cls, act, pose, q, ppose.reshape(B, NP, DP), pact)


def _make_runner(nc):
    """Build a reusable jitted SPMD runner (mirrors bass2jax.run_bass_via_pjrt,
    but keeps the jitted callable so repeat executions don't retrace)."""
    import jax
    from jax.sharding import Mesh, PartitionSpec
    from jax.experimental.shard_map import shard_map
    from concourse import bass2jax

    bass2jax.install_neuronx_cc_hook()
    partition_name = (nc.partition_id_tensor.name if nc.partition_id_tensor
                      else None)
    in_names, out_names, out_avals = [], [], []
    for alloc in nc.m.functions[0].allocations:
        if not isinstance(alloc, mybir.MemoryLocationSet):
            continue
        name = alloc.memorylocations[0].name
        if alloc.kind == "ExternalInput":
            if name != partition_name:
                in_names.append(name)
        elif alloc.kind == "ExternalOutput":
            out_names.append(name)
            out_avals.append(jax.core.ShapedArray(
                tuple(alloc.tensor_shape), mybir.dt.np(alloc.dtype)))
    n_params = len(in_names)
    all_in = in_names + out_names
    if partition_name is not None:
        all_in = all_in + [partition_name]
    donate = tuple(range(n_params, n_params + len(out_names)))

    def _body(*args):
        operands = list(args)
        if partition_name is not None:
            operands.append(bass2jax.partition_id_tensor())
        return tuple(bass2jax._bass_exec_p.bind(
            *operands, out_avals=tuple(out_avals), in_names=tuple(all_in),
            out_names=tuple(out_names), lowering_input_output_aliases=(),
            sim_require_finite=True, sim_require_nnan=True, nc=nc))

    devices = jax.devices()[:N_CORES]
    mesh = Mesh(np.asarray(devices), ("core",))
    nio = n_params + len(out_names)
    sharded = jax.jit(
        shard_map(_body, mesh=mesh, in_specs=(PartitionSpec("core"),) * nio,
                  out_specs=(PartitionSpec("core"),) * len(out_names),
                  check_rep=False),
        donate_argnums=donate, keep_unused=True)
    return sharded, in_names, out_names, out_avals, n_params


def _concat_inputs(in_maps, in_names):
    return [np.concatenate([np.asarray(in_maps[c][nm]) for c in range(N_CORES)],
                           axis=0) for nm in in_names]


def _fresh_zeros(out_avals):
    import jax
    return [jax.device_put(np.zeros((N_CORES * a.shape[0],) + a.shape[1:],
                                    a.dtype)) for a in out_avals]


def run_chained(inputs, chain=5):
    """Time `chain` back-to-back executions inside one jit call; the marginal
    per-execution time approximates true HW time (dispatch overhead amortized)."""
    import time
    import jax
    from jax.sharding import Mesh, PartitionSpec
    from jax.experimental.shard_map import shard_map
    from concourse import bass2jax

    nc = _get_program()
    in_maps = prepare_host(inputs)
    bass2jax.install_neuronx_cc_hook()
    partition_name = (nc.partition_id_tensor.name if nc.partition_id_tensor
                      else None)
    in_names, out_names, out_avals = [], [], []
    for alloc in nc.m.functions[0].allocations:
        if not isinstance(alloc, mybir.MemoryLocationSet):
            continue
        name = alloc.memorylocations[0].name
        if alloc.kind == "ExternalInput":
            if name != partition_name:
                in_names.append(name)
        elif alloc.kind == "ExternalOutput":
            out_names.append(name)
            out_avals.append(jax.core.ShapedArray(
                tuple(alloc.tensor_shape), mybir.dt.np(alloc.dtype)))
    n_params = len(in_names)
    all_in = in_names + out_names
    if partition_name is not None:
        all_in = all_in + [partition_name]

    def _body(*args):
        operands = list(args)
        if partition_name is not None:
            operands.append(bass2jax.partition_id_tensor())
        return tuple(bass2jax._bass_exec_p.bind(
            *operands, out_avals=tuple(out_avals), in_names=tuple(all_in),
            out_names=tuple(out_names), lowering_input_output_aliases=(),
            sim_require_finite=True, sim_require_nnan=True, nc=nc))

    def _chain_fn(n):
        def f(*args):
            ins = args[:n_params]
            outs = args[n_params:]
            for _ in range(n):
                outs = _body(*ins, *outs)
            return outs
        return f

    devices = jax.devices()[:N_CORES]
    mesh = Mesh(np.asarray(devices), ("core",))
    nio = n_params + len(out_names)
    concat_in = [jax.device_put(x) for x in _concat_inputs(in_maps, in_names)]
    jax.block_until_ready(concat_in)

    results = {}
    for n in (1, chain):
        fn = jax.jit(
            shard_map(_chain_fn(n), mesh=mesh,
                      in_specs=(PartitionSpec("core"),) * nio,
                      out_specs=(PartitionSpec("core"),) * len(out_names)),
            donate_argnums=tuple(range(n_params, nio)), keep_unused=True)
        ts = []
        for _ in range(4):
            zeros = _fresh_zeros(out_avals)
            jax.block_until_ready(zeros)
            t0 = time.perf_counter()
            cur = fn(*concat_in, *zeros)
            jax.block_until_ready(cur)
            ts.append((time.perf_counter() - t0) * 1e9)
        results[n] = min(ts[1:]) if len(ts) > 1 else ts[0]
    marginal = (results[chain] - results[1]) / (chain - 1)
    return marginal, results


def run_device(inputs, iters=1):
    """Run the kernel; returns (outputs_tuple, per-iteration wall ns list)."""
    import time
    import jax
    nc = _get_program()
    in_maps = prepare_host(inputs)
    sharded, in_names, out_names, out_avals, n_params = _make_runner(nc)
    concat_in = [jax.device_put(x) for x in _concat_inputs(in_maps, in_names)]
    jax.block_until_ready(concat_in)
    out_arrs = None
    times = []
    for _ in range(max(1, iters)):
        zeros = _fresh_zeros(out_avals)
        jax.block_until_ready(zeros)
        t0 = time.perf_counter()
        cur = sharded(*concat_in, *zeros)
        jax.block_until_ready(cur)
        times.append((time.perf_counter() - t0) * 1e9)
        out_arrs = cur
    results = []
    for c in range(N_CORES):
        results.append({nm: np.asarray(out_arrs[i]).reshape(
            (N_CORES,) + out_avals[i].shape)[c]
            for i, nm in enumerate(out_names)})
    return assemble(results), times


def kernel(**inputs):
    outs, _ = run_device(inputs, iters=1)
    return outs


if __name__ == "__main__":
    import reference as ref
    inputs = ref.setup_inputs()
    outs = kernel(**{k: np.asarray(v) for k, v in inputs.items()})
    print([o.shape for o in outs])


# revision 13
# speedup vs baseline: 145.9237x; 145.9237x over previous
"""Trainium2 Bass kernel for nn_PhenoCapsuleHead (capsule head with EM routing).

Contract: kernel(**inputs) takes the FULL inputs (B=4096) and returns the full
6-tuple (class_logits, act, pose, q, primary_pose, primary_act), matching
reference.reference(). Internally shards batch-parallel over 8 NeuronCores.

Design (per core, Bc=512, 4 partition-tiles of 128 samples):
  phase 1  : big matmul z_ext @ W_ext on PE (fp32r), where W_ext packs
             [W_prim | ln1_g*W_act | ones/D] and a ones-row folds in all biases.
             LayerNorm(z) @ W_act is computed analytically from (z@Wg, mean, var);
             sum(z^2) via ACT Square + PE ones-column matmuls.
  phase 1.5: EM init pose0 = einsum(a/25 * pose_p, W_caps) done entirely on PE.
  phase 2  : votes = pose_p @ W_caps per input-capsule (K=32 row-packed matmuls),
             EM routing on VectorE in fp16 (m padded 25->26 for 2x mode),
             with in-place tree reductions; exp/sigmoid on ScalarE.
"""
import os
import numpy as np

import concourse.bass as bass
import concourse.bacc as bacc
import concourse.tile as tile
from concourse import mybir
from concourse.bass_utils import run_bass_kernel_spmd

F32 = mybir.dt.float32
F32R = mybir.dt.float32r
F16 = mybir.dt.float16
AF = mybir.ActivationFunctionType
ALU = mybir.AluOpType
AX = mybir.AxisListType

N_CORES = 8
B = 4096
BC = B // N_CORES          # 512 per core
P = 128
NT = BC // P               # 4 partition tiles per core
D = 4096
KC = D // P                # 32 contraction chunks
NP = 64                    # primary capsules
DP = 32                    # primary dim
O = 16                     # class capsule dim
M = 25                     # n classes
MP = 26                    # padded (even, 4B-aligned fp16 runs)
OM = O * MP                # 416
NW = 2048 + NP + 4         # 2116 columns of W_ext (aligned for fp32r)
NCH = [512, 512, 512, 512, NW - 2048]   # N-chunks of the big matmul
EPS = 1e-8
LN_EPS = 1e-5
SCALE = float(1.0 / np.sqrt(O))

ROUTE16 = os.environ.get("ROUTE_F32", "0") != "1"   # fp16 routing by default
PHASE_LIMIT = os.environ.get("PHASE_LIMIT", "full")  # '1', '15', 'votes', 'full'
RDT = F16 if ROUTE16 else F32


def _bc(ap_obj, idx, count):
    """Insert a 0-step (broadcast) dim at position idx of an AP (0 = partition dim)."""
    new = [list(p) for p in ap_obj.ap]
    new.insert(idx, [0, count])
    return bass.AP(tensor=ap_obj.tensor, offset=ap_obj.offset, ap=new)


def _r(ap_obj):
    return ap_obj.bitcast(F32R)


def build_program():
    nc = bacc.Bacc("TRN2", target_bir_lowering=False, debug=False,
                   num_devices=N_CORES)

    zt = nc.dram_tensor("zt", [D + 1, BC], F32R, kind="ExternalInput").ap()
    wext = nc.dram_tensor("wext", [D + 1, NW], F32R, kind="ExternalInput").ap()
    wcaps = nc.dram_tensor("wcaps", [P, 16, OM], F16, kind="ExternalInput").ap()
    rrep = nc.dram_tensor("rrep", [NP, 16, P], F16, kind="ExternalInput").ap()
    identw = nc.dram_tensor("identw", [P, P], F32, kind="ExternalInput").ap()
    c1rep = nc.dram_tensor("c1rep", [P, NP], F32, kind="ExternalInput").ap()
    wcombrep = nc.dram_tensor("wcombrep", [P, O], F32, kind="ExternalInput").ap()
    cbeta = nc.dram_tensor("cbeta", [P, 1], F32, kind="ExternalInput").ap()

    o_ppose = nc.dram_tensor("ppose", [BC, 2048], F32, kind="ExternalOutput").ap()
    o_pact = nc.dram_tensor("pact", [BC, NP], F32, kind="ExternalOutput").ap()
    o_q = nc.dram_tensor("q", [BC, NP, M], F32, kind="ExternalOutput").ap()
    o_pose = nc.dram_tensor("pose", [BC, M, O], F32, kind="ExternalOutput").ap()
    o_act = nc.dram_tensor("act", [BC, M], F32, kind="ExternalOutput").ap()
    o_cls = nc.dram_tensor("cls", [BC, M], F32, kind="ExternalOutput").ap()

    with tile.TileContext(nc) as tc:
        _build(nc, tc, zt, wext, wcaps, rrep, identw, c1rep, wcombrep, cbeta,
               o_ppose, o_pact, o_q, o_pose, o_act, o_cls)
    nc.compile()
    return nc


def _build(nc, tc, zt, wext, wcaps, rrep, identw, c1rep, wcombrep, cbeta,
           o_ppose, o_pact, o_q, o_pose, o_act, o_cls):
    from contextlib import ExitStack
    ctx = ExitStack()
    with ctx:
        const = ctx.enter_context(tc.tile_pool(name="const", bufs=1))
        persist = ctx.enter_context(tc.tile_pool(name="persist", bufs=1))
        smalls = ctx.enter_context(tc.tile_pool(name="smalls", bufs=4))

        # ---------------- constants ----------------
        wcaps_s = const.tile([P, 16, OM], F16)
        nc.sync.dma_start(out=wcaps_s, in_=wcaps)
        rrep_s = const.tile([NP, 16, P], F16)
        nc.sync.dma_start(out=rrep_s, in_=rrep)
        ident_s = const.tile([P, P], F32)
        nc.sync.dma_start(out=ident_s, in_=identw)
        c1rep_s = const.tile([P, NP], F32)
        nc.sync.dma_start(out=c1rep_s, in_=c1rep)
        wcomb_s = const.tile([P, O], F32)
        nc.sync.dma_start(out=wcomb_s, in_=wcombrep)
        cbeta_s = const.tile([P, 1], F32)
        nc.sync.dma_start(out=cbeta_s, in_=cbeta)
        ones_col = const.tile([P, 1], F32)
        nc.vector.memset(ones_col, 1.0)
        lneps_t = const.tile([P, 1], F32)
        nc.vector.memset(lneps_t, LN_EPS)
        ztb = const.tile([1, BC], F32R)
        nc.sync.dma_start(out=ztb, in_=zt[D:D + 1, :])

        # ---------------- persistent state ----------------
        pose_pT = persist.tile([P, 16, BC], F16)       # [nd, c, b] 16KB/part
        a_full = persist.tile([P, NT, NP], F32)        # primary_act per tile
        zwg_s = persist.tile([P, NT, NP], F32)
        mu_s = persist.tile([P, NT, 1], F32)
        rr_s = persist.tile([P, NT, 1], F32)
        aT_s = persist.tile([NP, BC], F16)             # a transposed [n, b]
        pose_f32 = [persist.tile([P, O, MP], F32, tag=f"pose{t}", name=f"pose_f32_{t}")
                    for t in range(NT)]

        # ================= phase 1: big matmul =================
        with tc.tile_pool(name="ztp", bufs=1) as ztp, \
             tc.tile_pool(name="stream", bufs=3) as stream, \
             tc.tile_pool(name="zsqp", bufs=16) as zsqp, \
             tc.tile_pool(name="stage1", bufs=1) as stage1:
            zt_s = ztp.tile([P, KC, BC], F32R)          # 64KB/part, freed after ph.1
            for kc in range(KC):
                nc.sync.dma_start(out=zt_s[:, kc, :], in_=zt[P * kc:P * (kc + 1), :])
            pose_ps = stage1.tile([P, NT, 2048], F32)   # 32KB/part staging

            with tc.tile_pool(name="bigp", bufs=2, space="PSUM") as bigp:
                ncol0 = 0
                for nci, nsz in enumerate(NCH):
                    bps = [bigp.tile([P, 512], F32, tag=f"big{bt}", name=f"bps{bt}")
                           for bt in range(NT)]
                    for kc in range(KC):
                        wt = stream.tile([P, 512], F32R, tag="wt")
                        nc.sync.dma_start(
                            out=wt[:, 0:nsz],
                            in_=wext[P * kc:P * (kc + 1), ncol0:ncol0 + nsz])
                        for bt in range(NT):
                            nc.tensor.matmul(bps[bt][:, 0:nsz],
                                             zt_s[:, kc, P * bt:P * (bt + 1)],
                                             wt[:, 0:nsz],
                                             start=(kc == 0), stop=False)
                    wb = stream.tile([1, 512], F32R, tag="wb")
                    nc.sync.dma_start(out=wb[:, 0:nsz],
                                      in_=wext[D:D + 1, ncol0:ncol0 + nsz])
                    for bt in range(NT):
                        nc.tensor.matmul(bps[bt][:, 0:nsz],
                                         ztb[:, P * bt:P * (bt + 1)],
                                         wb[:, 0:nsz], start=False, stop=True)
                    for bt in range(NT):
                        if nci < 4:
                            nc.scalar.activation(
                                out=pose_ps[:, bt, ncol0:ncol0 + nsz],
                                in_=bps[bt], func=AF.Copy)
                            nc.sync.dma_start(
                                out=o_ppose[P * bt:P * (bt + 1), ncol0:ncol0 + nsz],
                                in_=pose_ps[:, bt, ncol0:ncol0 + nsz])
                        else:
                            nc.scalar.activation(out=zwg_s[:, bt, :],
                                                 in_=bps[bt][:, 0:NP], func=AF.Copy)
                            nc.vector.tensor_copy(mu_s[:, bt, :],
                                                  bps[bt][:, NP:NP + 1])
                    ncol0 += nsz

            # ---- pose_p transposes + sum(z^2) + primary_act (bigp freed) ----
            with tc.tile_pool(name="ssp", bufs=1, space="PSUM") as ssp, \
                 tc.tile_pool(name="tp_ps", bufs=2, space="PSUM") as tp_ps, \
                 tc.tile_pool(name="tp_ps2", bufs=2, space="PSUM") as tp_ps2:
                for cc in range(16):
                    for bt in range(NT):
                        tp = tp_ps.tile([P, P], F32, tag="tp")
                        nc.tensor.transpose(
                            tp, pose_ps[:, bt, P * cc:P * (cc + 1)], ident_s)
                        nc.vector.tensor_copy(
                            pose_pT[:, cc, P * bt:P * (bt + 1)], tp)
                ssb = [ssp.tile([P, 1], F32, tag=f"ss{bt}", name=f"ssb{bt}") for bt in range(NT)]
                for kc in range(KC):
                    zsq = zsqp.tile([P, 512], F32, tag="zsq")
                    nc.scalar.activation(out=zsq, in_=zt_s[:, kc, :].bitcast(F32),
                                         func=AF.Square)
                    for bt in range(NT):
                        nc.tensor.matmul(ssb[bt], zsq[:, P * bt:P * (bt + 1)],
                                         ones_col, start=(kc == 0),
                                         stop=(kc == KC - 1))
                for bt in range(NT):
                    musq = smalls.tile([P, 1], F32, tag="musq")
                    nc.vector.tensor_mul(musq, mu_s[:, bt, :], mu_s[:, bt, :])
                    var = smalls.tile([P, 1], F32, tag="var")
                    nc.vector.scalar_tensor_tensor(
                        out=var, in0=ssb[bt], scalar=1.0 / D, in1=musq,
                        op0=ALU.mult, op1=ALU.subtract)
                    sd = smalls.tile([P, 1], F32, tag="sd")
                    nc.scalar.activation(out=sd, in_=var, func=AF.Sqrt, bias=lneps_t)
                    nc.vector.reciprocal(rr_s[:, bt, :], sd)
                    rmun = smalls.tile([P, 1], F32, tag="rmun")
                    nc.vector.tensor_mul(rmun, rr_s[:, bt, :], mu_s[:, bt, :])
                    nc.vector.tensor_scalar_mul(rmun, rmun, -1.0)
                    actl = smalls.tile([P, NP], F32, tag="actl")
                    nc.vector.tensor_scalar_mul(actl, zwg_s[:, bt, :], rr_s[:, bt, :])
                    nc.vector.scalar_tensor_tensor(
                        out=actl, in0=c1rep_s, scalar=rmun, in1=actl,
                        op0=ALU.mult, op1=ALU.add)
                    nc.scalar.activation(out=a_full[:, bt, :], in_=actl,
                                         func=AF.Sigmoid)
                    nc.sync.dma_start(out=o_pact[P * bt:P * (bt + 1), :],
                                      in_=a_full[:, bt, :])
                    tpa = tp_ps2.tile([NP, P], F32, tag="tpa")
                    nc.tensor.transpose(tpa, a_full[:, bt, :], ident_s)
                    nc.scalar.activation(out=aT_s[:, P * bt:P * (bt + 1)], in_=tpa,
                                         func=AF.Copy)

        if PHASE_LIMIT == "1":
            return
        # ================= phase 1.5: init pose0 via PE =================
        with tc.tile_pool(name="p15", bufs=3) as p15, \
             tc.tile_pool(name="p15ps", bufs=2, space="PSUM") as p15ps, \
             tc.tile_pool(name="p0acc", bufs=1, space="PSUM") as p0acc:
            omch = [(0, 128), (128, 128), (256, 128), (384, OM - 384)]
            pacc = [p0acc.tile([oc[1], BC], F32, tag=f"p0{j}", name=f"pacc{j}")
                    for j, oc in enumerate(omch)]
            for c16 in range(16):
                rep = p15ps.tile([P, BC], F32, tag="rep")
                nc.tensor.matmul(rep, rrep_s[:, c16, :], aT_s,
                                 start=True, stop=True)
                aps = p15.tile([P, BC], F16, tag="aps")
                nc.vector.tensor_mul(aps, pose_pT[:, c16, :], rep)
                for j, (o0, osz) in enumerate(omch):
                    nc.tensor.matmul(pacc[j], wcaps_s[:, c16, o0:o0 + osz],
                                     aps, start=(c16 == 0), stop=(c16 == 15))
            psts = []
            for j, (o0, osz) in enumerate(omch):
                pst = p15.tile([P, BC], F32, tag=f"pst{j}", name=f"pst{j}")
                nc.scalar.activation(out=pst[0:osz, :], in_=pacc[j], func=AF.Copy)
                psts.append(pst)
            for bt in range(NT):
                for j, (o0, osz) in enumerate(omch):
                    tp = p15ps.tile([P, P], F32, tag="tp15")
                    nc.tensor.transpose(tp[:, 0:osz],
                                        psts[j][0:osz, P * bt:P * (bt + 1)],
                                        ident_s[0:osz, 0:osz])
                    pf = pose_f32[bt]
                    nc.scalar.activation(
                        out=bass.AP(tensor=pf.tensor, offset=pf.offset + o0,
                                    ap=[list(pf.ap[0]), [1, osz]]),
                        in_=tp[:, 0:osz], func=AF.Copy)

        if PHASE_LIMIT == "15":
            return
        # ================= phase 2: votes + routing =================
        # V and tmp are split in n-halves over a 5-slot rotation so the next
        # tile's votes copies overlap the current tile's routing.
        with tc.tile_pool(name="bigbuf", bufs=5) as bigbuf, \
             tc.tile_pool(name="rt", bufs=1) as rt, \
             tc.tile_pool(name="rt2", bufs=2) as rt2, \
             tc.tile_pool(name="vps", bufs=2, space="PSUM") as vps:
            HN = NP // 2
            for bt in range(NT):
                # ---- votes ----
                Va = bigbuf.tile([P, HN, O, MP], RDT, tag="hv", name=f"Va{bt}")
                Vb = bigbuf.tile([P, HN, O, MP], RDT, tag="hv", name=f"Vb{bt}")
                for c16 in range(16):
                    ps = vps.tile([P, 4, 512], F32, tag="v")
                    for r in range(4):
                        nc.tensor.matmul(
                            ps[:, r, 0:OM],
                            pose_pT[DP * r:DP * (r + 1), c16, P * bt:P * (bt + 1)],
                            wcaps_s[DP * r:DP * (r + 1), c16, :],
                            start=True, stop=True, tile_position=(DP * r, 0))
                    half = Va if c16 < 8 else Vb
                    c4 = 4 * (c16 % 8)
                    nc.scalar.activation(
                        out=half[:, c4:c4 + 4, :, :].rearrange(
                            "p a o m -> p a (o m)"),
                        in_=ps[:, :, 0:OM], func=AF.Copy)

                if PHASE_LIMIT == "votes":
                    continue
                # ---- routing state ----
                a_t = a_full[:, bt, :]
                actE = rt.tile([P, MP], F32, tag="actE")
                suma = smalls.tile([P, 1], F32, tag="suma")
                nc.vector.reduce_sum(suma, a_t, axis=AX.X)
                act01 = smalls.tile([P, 1], F32, tag="act01")
                nc.scalar.activation(out=act01, in_=suma, func=AF.Sigmoid,
                                     scale=1.0 / M)
                nc.vector.tensor_scalar_add(
                    actE, bass.AP(tensor=act01.tensor, offset=act01.offset,
                                  ap=[list(act01.ap[0]), [0, MP]]), EPS)
                nc.vector.memset(actE[:, M:MP], 0.0)

                ta = bigbuf.tile([P, HN, O, MP], RDT, tag="hv", name=f"ta{bt}")
                tb = bigbuf.tile([P, HN, O, MP], RDT, tag="hv", name=f"tb{bt}")
                posef = rt.tile([P, O, MP], RDT, tag="posef")
                if ROUTE16:
                    nc.vector.tensor_copy(posef, pose_f32[bt])
                e_s = rt2.tile([P, NP, MP], F32, tag="e")
                qsum = rt.tile([P, MP], F32, tag="qsum")
                act_s = rt2.tile([P, MP], F32, tag="acts")
                Z = rt.tile([P, NP], F32, tag="Z")
                rz = rt.tile([P, NP], F32, tag="rz")

                for it in range(2):
                    psrc = posef if ROUTE16 else pose_f32[bt]
                    dot = rt.tile([P, NP, MP], RDT, tag="dq")
                    for V, tmp, n0 in ((Va, ta, 0), (Vb, tb, HN)):
                        nc.vector.tensor_mul(tmp, V, _bc(psrc, 1, HN))
                        nc.vector.tensor_add(tmp[:, :, 0:8, :], tmp[:, :, 0:8, :],
                                             tmp[:, :, 8:16, :])
                        nc.vector.tensor_add(tmp[:, :, 0:4, :], tmp[:, :, 0:4, :],
                                             tmp[:, :, 4:8, :])
                        nc.vector.tensor_add(tmp[:, :, 0:2, :], tmp[:, :, 0:2, :],
                                             tmp[:, :, 2:4, :])
                        nc.vector.tensor_add(dot[:, n0:n0 + HN, :],
                                             tmp[:, :, 0, :], tmp[:, :, 1, :])
                    nc.scalar.activation(out=e_s, in_=dot, func=AF.Exp, scale=SCALE)
                    nc.vector.tensor_mul(e_s, e_s, _bc(actE, 1, NP))
                    nc.vector.reduce_sum(Z, e_s, axis=AX.X)
                    nc.vector.reciprocal(rz, Z)
                    nc.vector.tensor_mul(e_s, e_s, _bc(rz, 2, MP))   # q (f32)
                    qa = rt.tile([P, NP, MP], RDT, tag="dq")
                    nc.vector.tensor_mul(qa, e_s, _bc(a_t, 2, MP))
                    nc.vector.reduce_sum(qsum, qa.rearrange("p n m -> p m n"),
                                         axis=AX.X)
                    nc.scalar.activation(out=act_s, in_=qsum, func=AF.Sigmoid)
                    if it == 0:
                        nc.vector.tensor_scalar_add(actE, act_s, EPS)
                        nc.vector.memset(actE[:, M:MP], 0.0)
                    nc.vector.tensor_mul(ta, Va, _bc(qa[:, 0:HN, :], 2, O))
                    nc.vector.tensor_mul(tb, Vb, _bc(qa[:, HN:NP, :], 2, O))
                    nc.vector.tensor_add(ta, ta, tb)
                    nc.vector.tensor_add(ta[:, 0:16], ta[:, 0:16], ta[:, 16:32])
                    nc.vector.tensor_add(ta[:, 0:8], ta[:, 0:8], ta[:, 8:16])
                    nc.vector.tensor_add(ta[:, 0:4], ta[:, 0:4], ta[:, 4:8])
                    nc.vector.tensor_add(ta[:, 0:2], ta[:, 0:2], ta[:, 2:4])
                    nc.vector.tensor_add(pose_f32[bt], ta[:, 0], ta[:, 1])
                    if it == 0 and ROUTE16:
                        nc.vector.tensor_copy(posef, pose_f32[bt])

                # ---- outputs of routing ----
                nc.sync.dma_start(out=o_q[P * bt:P * (bt + 1), :, :],
                                  in_=e_s[:, :, 0:M])
                nc.sync.dma_start(out=o_act[P * bt:P * (bt + 1), :],
                                  in_=act_s[:, 0:M])
                pmo = rt2.tile([P, M, O], F32, tag="pmo")
                nc.vector.tensor_copy(
                    pmo.rearrange("p m o -> p o m"), pose_f32[bt][:, :, 0:M])
                nc.sync.dma_start(out=o_pose[P * bt:P * (bt + 1), :, :], in_=pmo)

                # ---- class logits: LN over o ----
                pf = pose_f32[bt]
                mu2 = rt.tile([P, MP], F32, tag="mu2")
                nc.vector.tensor_reduce(mu2, pf.rearrange("p o m -> p m o"),
                                        axis=AX.X, op=ALU.add)
                dctr = rt.tile([P, O, MP], F32, tag="dctr")
                nc.vector.scalar_tensor_tensor(
                    out=dctr, in0=_bc(mu2, 1, O), scalar=-1.0 / O, in1=pf,
                    op0=ALU.mult, op1=ALU.add)
                sq2 = rt.tile([P, O, MP], F32, tag="sqtw")
                nc.vector.tensor_mul(sq2, dctr, dctr)
                v2 = rt.tile([P, MP], F32, tag="v2")
                nc.vector.tensor_reduce(v2, sq2.rearrange("p o m -> p m o"),
                                        axis=AX.X, op=ALU.add)
                sd2 = rt.tile([P, MP], F32, tag="sd2")
                nc.scalar.activation(out=sd2, in_=v2, func=AF.Sqrt, bias=lneps_t,
                                     scale=1.0 / O)
                r2 = rt.tile([P, MP], F32, tag="r2")
                nc.vector.reciprocal(r2, sd2)
                tw = rt.tile([P, O, MP], F32, tag="sqtw")
                nc.vector.tensor_mul(tw, dctr, _bc(wcomb_s, 2, MP))
                inner = rt.tile([P, MP], F32, tag="inner")
                nc.vector.tensor_reduce(inner, tw.rearrange("p o m -> p m o"),
                                        axis=AX.X, op=ALU.add)
                cls = rt2.tile([P, MP], F32, tag="cls")
                nc.vector.tensor_mul(cls, inner, r2)
                nc.vector.tensor_scalar_add(
                    cls, cls, bass.AP(tensor=cbeta_s.tensor, offset=cbeta_s.offset,
                                      ap=[list(cbeta_s.ap[0]), [1, 1]]))
                nc.sync.dma_start(out=o_cls[P * bt:P * (bt + 1), :], in_=cls[:, 0:M])


def prepare_host(inputs):
    """Host-side weight preparation (shared across cores)."""
    z = np.ascontiguousarray(np.asarray(inputs["z"], dtype=np.float32))
    W_prim = np.asarray(inputs["W_prim"], np.float32)
    b_prim = np.asarray(inputs["b_prim"], np.float32)
    ln1_g = np.asarray(inputs["ln1_g"], np.float32)
    ln1_b = np.asarray(inputs["ln1_b"], np.float32)
    W_act = np.asarray(inputs["W_act"], np.float32)
    b_act = np.asarray(inputs["b_act"], np.float32)
    W_caps = np.asarray(inputs["W_caps"], np.float32)
    ln2_g = np.asarray(inputs["ln2_g"], np.float32)
    ln2_b = np.asarray(inputs["ln2_b"], np.float32)
    W_out = np.asarray(inputs["W_out"], np.float32)
    b_out = np.asarray(inputs["b_out"], np.float32)

    W_ext = np.zeros((D + 1, NW), np.float32)
    W_ext[:D, :2048] = W_prim
    W_ext[:D, 2048:2112] = ln1_g[:, None] * W_act
    W_ext[:D, 2112] = 1.0 / D
    W_ext[D, :2048] = b_prim
    W_ext[D, 2048:2112] = ln1_b @ W_act + b_act
    W_ext[D, 2112] = 0.0

    W_caps2 = np.zeros((NP, DP, O, MP), np.float32)
    W_caps2[:, :, :, :M] = np.transpose(W_caps, (0, 1, 3, 2))
    wcaps_arr = np.ascontiguousarray(
        W_caps2.reshape(16, 4, DP, OM).transpose(1, 2, 0, 3).reshape(P, 16, OM)
    ).astype(np.float16)

    rrep_arr = np.zeros((NP, 16, P), np.float16)
    for n in range(NP):
        c, r = n // 4, n % 4
        rrep_arr[n, c, DP * r:DP * (r + 1)] = 1.0 / M

    ident = np.eye(P, dtype=np.float32)
    c1 = W_act.T @ ln1_g
    c1rep_arr = np.ascontiguousarray(np.broadcast_to(c1, (P, NP)))
    wcomb = ln2_g * W_out[:, 0]
    wcomb_arr = np.ascontiguousarray(np.broadcast_to(wcomb, (P, O)))
    cb = np.float32(ln2_b @ W_out[:, 0] + b_out[0])
    cbeta_arr = np.full((P, 1), cb, np.float32)

    zt_full = np.concatenate([z.T, np.ones((1, B), np.float32)], axis=0)  # [4097, B]
    shared = dict(wext=W_ext, wcaps=wcaps_arr, rrep=rrep_arr, identw=ident,
                  c1rep=c1rep_arr, wcombrep=wcomb_arr, cbeta=cbeta_arr)
    in_maps = []
    for c in range(N_CORES):
        m = dict(shared)
        m["zt"] = np.ascontiguousarray(zt_full[:, c * BC:(c + 1) * BC])
        in_maps.append(m)
    return in_maps


_CACHE = {}


def _get_program():
    key = ("nc_f16" if ROUTE16 else "nc_f32") + PHASE_LIMIT
    if key not in _CACHE:
        _CACHE[key] = build_program()
    return _CACHE[key]


def assemble(results):
    cls = np.concatenate([r["cls"] for r in results], axis=0)
    act = np.concatenate([r["act"] for r in results], axis=0)
    pose = np.concatenate([r["pose"] for r in results], axis=0)
    q = np.concatenate([r["q"] for r in results], axis=0)
    ppose = np.concatenate([r["ppose"] for r in results], axis=0)
    pact = np.concatenate([r["pact"] for r in results], axis=0)
    return (cls, act, pose, q, ppose.reshape(B, NP, DP), pact)


def _make_runner(nc):
    """Build a reusable jitted SPMD runner (mirrors bass2jax.run_bass_via_pjrt,
    but keeps the jitted callable so repeat executions don't retrace)."""
    import jax
    from jax.sharding import Mesh, PartitionSpec
    from jax.experimental.shard_map import shard_map
    from concourse import bass2jax

    bass2jax.install_neuronx_cc_hook()
    partition_name = (nc.partition_id_tensor.name if nc.partition_id_tensor
                      else None)
    in_names, out_names, out_avals = [], [], []
    for alloc in nc.m.functions[0].allocations:
        if not isinstance(alloc, mybir.MemoryLocationSet):
            continue
        name = alloc.memorylocations[0].name
        if alloc.kind == "ExternalInput":
            if name != partition_name:
                in_names.append(name)
        elif alloc.kind == "ExternalOutput":
            out_names.append(name)
            out_avals.append(jax.core.ShapedArray(
                tuple(alloc.tensor_shape), mybir.dt.np(alloc.dtype)))
    n_params = len(in_names)
    all_in = in_names + out_names
    if partition_name is not None:
        all_in = all_in + [partition_name]
    donate = tuple(range(n_params, n_params + len(out_names)))

    def _body(*args):
        operands = list(args)
        if partition_name is not None:
            operands.append(bass2jax.partition_id_tensor())
        return tuple(bass2jax._bass_exec_p.bind(
            *operands, out_avals=tuple(out_avals), in_names=tuple(all_in),
            out_names=tuple(out_names), lowering_input_output_aliases=(),
            sim_require_finite=True, sim_require_nnan=True, nc=nc))

    devices = jax.devices()[:N_CORES]
    mesh = Mesh(np.asarray(devices), ("core",))
    nio = n_params + len(out_names)
    sharded = jax.jit(
        shard_map(_body, mesh=mesh, in_specs=(PartitionSpec("core"),) * nio,
                  out_specs=(PartitionSpec("core"),) * len(out_names),
                  check_rep=False),
        donate_argnums=donate, keep_unused=True)
    return sharded, in_names, out_names, out_avals, n_params


def _concat_inputs(in_maps, in_names):
    return [np.concatenate([np.asarray(in_maps[c][nm]) for c in range(N_CORES)],
                           axis=0) for nm in in_names]


def _fresh_zeros(out_avals):
    import jax
    return [jax.device_put(np.zeros((N_CORES * a.shape[0],) + a.shape[1:],
                                    a.dtype)) for a in out_avals]


def run_chained(inputs, chain=5):
    """Time `chain` back-to-back executions inside one jit call; the marginal
    per-execution time approximates true HW time (dispatch overhead amortized)."""
    import time
    import jax
    from jax.sharding import Mesh, PartitionSpec
    from jax.experimental.shard_map import shard_map
    from concourse import bass2jax

    nc = _get_program()
    in_maps = prepare_host(inputs)
    bass2jax.install_neuronx_cc_hook()
    partition_name = (nc.partition_id_tensor.name if nc.partition_id_tensor
                      else None)
    in_names, out_names, out_avals = [], [], []
    for alloc in nc.m.functions[0].allocations:
        if not isinstance(alloc, mybir.MemoryLocationSet):
            continue
        name = alloc.memorylocations[0].name
        if alloc.kind == "ExternalInput":
            if name != partition_name:
                in_names.append(name)
        elif alloc.kind == "ExternalOutput":
            out_names.append(name)
            out_avals.append(jax.core.ShapedArray(
                tuple(alloc.tensor_shape), mybir.dt.np(alloc.dtype)))
    n_params = len(in_names)
    all_in = in_names + out_names
    if partition_name is not None:
        all_in = all_in + [partition_name]

    def _body(*args):
        operands = list(args)
        if partition_name is not None:
            operands.append(bass2jax.partition_id_tensor())
        return tuple(bass2jax._bass_exec_p.bind(
            *operands, out_avals=tuple(out_avals), in_names=tuple(all_in),
            out_names=tuple(out_names), lowering_input_output_aliases=(),
            sim_require_finite=True, sim_require_nnan=True, nc=nc))

    def _chain_fn(n):
        def f(*args):
            ins = args[:n_params]
            outs = args[n_params:]
            for _ in range(n):
                outs = _body(*ins, *outs)
            return outs
        return f

    devices = jax.devices()[:N_CORES]
    mesh = Mesh(np.asarray(devices), ("core",))
    nio = n_params + len(out_names)
    concat_in = [jax.device_put(x) for x in _concat_inputs(in_maps, in_names)]
    jax.block_until_ready(concat_in)

    results = {}
    for n in (1, chain):
        fn = jax.jit(
            shard_map(_chain_fn(n), mesh=mesh,
                      in_specs=(PartitionSpec("core"),) * nio,
                      out_specs=(PartitionSpec("core"),) * len(out_names)),
            donate_argnums=tuple(range(n_params, nio)), keep_unused=True)
        ts = []
        for _ in range(4):
            zeros = _fresh_zeros(out_avals)
            jax.block_until_ready(zeros)
            t0 = time.perf_counter()
            cur = fn(*concat_in, *zeros)
            jax.block_until_ready(cur)
            ts.append((time.perf_counter() - t0) * 1e9)
        results[n] = min(ts[1:]) if len(ts) > 1 else ts[0]
    marginal = (results[chain] - results[1]) / (chain - 1)
    return marginal, results


def run_device(inputs, iters=1):
    """Run the kernel; returns (outputs_tuple, per-iteration wall ns list)."""
    import time
    import jax
    nc = _get_program()
    in_maps = prepare_host(inputs)
    sharded, in_names, out_names, out_avals, n_params = _make_runner(nc)
    concat_in = [jax.device_put(x) for x in _concat_inputs(in_maps, in_names)]
    jax.block_until_ready(concat_in)
    out_arrs = None
    times = []
    for _ in range(max(1, iters)):
        zeros = _fresh_zeros(out_avals)
        jax.block_until_ready(zeros)
        t0 = time.perf_counter()
        cur = sharded(*concat_in, *zeros)
        jax.block_until_ready(cur)
        times.append((time.perf_counter() - t0) * 1e9)
        out_arrs = cur
    results = []
    for c in range(N_CORES):
        results.append({nm: np.asarray(out_arrs[i]).reshape(
            (N_CORES,) + out_avals[i].shape)[c]
            for i, nm in enumerate(out_names)})
    return assemble(results), times


def kernel(**inputs):
    outs, _ = run_device(inputs, iters=1)
    return outs


if __name__ == "__main__":
    import reference as ref
    inputs = ref.setup_inputs()
    outs = kernel(**{k: np.asarray(v) for k, v in inputs.items()})
    print([o.shape for o in outs])
